# revision 10
# baseline (speedup 1.0000x reference)
"""Trainium2 Bass kernel for DenseFeatureExtractionModule (irregular-pooled VGG).

Sharding: 8 cores = 2 images x 4 row-strips of the 192-grid output (48 rows
each). Each core receives its input strip with enough halo rows to compute
all 10 conv layers locally (no inter-core communication). Out-of-image halo
rows are kept at zero through the layer stack by per-row scale/bias data
(L1) or by multiplying edge-band rows with a per-core row-validity mask,
which reproduces SAME-conv zero padding.

Design (from 6.19 ms baseline -> 2.87 ms -> this):
- All activations/weights fp16 (PSUM fp32, output fp16): halves DMA and
  16-bit DVE costs at the same PE rate; rel err ~1.5e-3 (gate 2e-2).
- L1+L2 fused with A1 resident in SBUF (no DRAM roundtrip, no misaligned
  column-shift DMAs); L1 activations split Scalar/Vector with per-row
  scale+bias APs so invalid rows come out exactly zero without GpSimd
  per-row multiplies (which serialized the old pipeline).
- K<128 matmuls never warm the PE HAM clock gate (observed: whole layers
  running at 1.2 GHz). L2 and L3 therefore K-pack taps to 128: A1/A2 are
  stored twice (partitions 64-127 column-shifted by 1), so each row-tap
  needs one K=128 matmul for the (b=-1,b=0) pair plus one zero-padded
  K=128 matmul for b=+1.
- Every graph-conv layer (L5-L10) runs a SINGLE set of matmuls over
  per-tap mask-gathered inputs: a [p,gq,192] gathered tile per (ci, tap)
  serves gq/2 row-pairs; the center tap is dilation-invariant and needs no
  gather. Odd-column taps are copied on the Scalar engine; even-column
  taps as int32-pair DVE copies. The dilation-s overwrite is one DVE
  copy_predicated on int32 pairs with a half-resolution mask (legal since
  mask blocks are 4 pixels wide). Row-validity masking runs on GpSimd.
- Activations (PSUM->fp16 relu+bias) are emitted one pair late so gather
  copies never queue behind PE-dependent work on the same engine.
- DMA ring split: input loads issue on the Sync HWDGE ring, output stores
  on the Scalar ring, so next-layer loads never queue behind this layer's
  stores. Weight loads are emitted one layer early (land mid-layer).
- Output tiles carry their zero pad columns and are written full-width in
  one contiguous DMA per (block, co): no DRAM pad-zeroing pass.
"""

import numpy as np

import concourse.bacc as bacc
import concourse.bass as bass
import concourse.mybir as mybir
import concourse.tile as tile
from concourse.bass_utils import run_bass_kernel_spmd

F32 = mybir.dt.float32
F16 = mybir.dt.float16
I32 = mybir.dt.int32
U8 = mybir.dt.uint8
RELU = mybir.ActivationFunctionType.Relu
COPY = mybir.ActivationFunctionType.Copy
MAX = mybir.AluOpType.max
ADD = mybir.AluOpType.add

W192 = 192
W384 = 384
PAD = 4  # pad columns for all 192-grid buffers
WP = W192 + 2 * PAD  # 200
A1_WP = W384 + 2  # 386, pad 1

# buffer row counts (per core strips, incl. halo)
CANVAS_ROWS = 180  # batch strip on 384 grid (96 + 2*42)
A1_ROWS = 178
ROWS192 = {"A2": 88, "A3": 86, "A4": 84, "A5": 80, "A6": 76, "A7": 72,
           "A8": 64, "A9": 56, "OUT": 48}
HALO192 = {"A2": 20, "A3": 19, "A4": 18, "A5": 16, "A6": 14, "A7": 12,
           "A8": 8, "A9": 4, "OUT": 0}

_CHANS = [(3, 64), (64, 64), (64, 128), (128, 128), (128, 256),
          (256, 256), (256, 256), (256, 512), (512, 512), (512, 512)]

# (src, dst, Cin, Cout, dils, pool_s, widx) for layers 3..10
LAYERS = [
    ("A2", "A3", 64, 128, (1,), None, 3),
    ("A3", "A4", 128, 128, (1,), 2, 4),
    ("A4", "A5", 128, 256, (1, 2), None, 5),
    ("A5", "A6", 256, 256, (1, 2), None, 6),
    ("A6", "A7", 256, 256, (1, 2), 4, 7),
    ("A7", "A8", 256, 512, (1, 4), None, 8),
    ("A8", "A9", 512, 512, (1, 4), None, 9),
    ("A9", "OUT", 512, 512, (1, 4), None, 10),
]
RB = 8  # output rows per input-tile block (192-grid layers)

TAPS = [(a, b) for a in (-1, 0, 1) for b in (-1, 0, 1)]


def _emit_l12(nc, tc, pools, bufs, params):
    """Fused L1 (1x1 conv over im2col, K=27, M=64, 384 grid) and L2
    (64->64 conv + 2x2 maxpool -> A2, 192 grid), with A1 resident in SBUF.

    A1 is written directly by the activation ops (partitions 0-63); the
    K-pack copy (partitions 64-127, column-shifted by 1) is one SBUF->SBUF
    DMA per block. L1 activations split 50/50 Scalar/Vector; rows that can
    fall outside the image run on Scalar with per-row scale+bias APs
    (exact zero for invalid rows); Vector rows in edge blocks get a
    batched validity multiply split between Vector and GpSimd."""
    x1 = bufs["X1"]
    wsb = pools["const"].tile([27, 64], F16)
    nc.sync.dma_start(wsb[:], params["w1"][:])
    bsb = pools["const"].tile([64, 1], F32)
    nc.sync.dma_start(bsb[:], params["b1"][:])
    actb = pools["const"].tile([64, A1_ROWS], F32)  # b1 * rowvalid
    nc.sync.dma_start(actb[:], params["b1r"][:])
    acts = pools["const"].tile([64, A1_ROWS], F32)  # rowvalid
    nc.sync.dma_start(acts[:], params["rm384f"][:])
    rm = pools["const"].tile([64, A1_ROWS], F16)  # rowvalid f16 (for muls)
    nc.sync.dma_start(rm[:], params["rm384h"][:])

    a1 = pools["a1"].tile([128, A1_ROWS, A1_WP], F16, name="a1sb", tag="a1sb")
    # pad columns: col 0 (low), col 385 (low+high) stay zero throughout
    nc.gpsimd.memset(a1[0:64, :, 0:1], 0.0)
    nc.gpsimd.memset(a1[:, :, A1_WP - 1 : A1_WP], 0.0)

    RB1 = 16
    with tc.tile_pool(name="in1", bufs=3) as p_in1:
     for i0 in range(0, A1_ROWS, RB1):
        nr = min(RB1, A1_ROWS - i0)
        xt = p_in1.tile([27, RB1, W384], F16, tag="x1t")
        nc.sync.dma_start(xt[:, : nr // 2, :], x1[:, i0 : i0 + nr // 2, :])
        if nr > nr // 2:
            nc.sync.dma_start(xt[:, nr // 2 : nr, :],
                              x1[:, i0 + nr // 2 : i0 + nr, :])
        edge = i0 < 41 or i0 + nr > 137
        vrows = []
        for j in range(nr):
            i = i0 + j
            ps = pools["psum"].tile([64, W384], F32, tag="ps")
            nc.tensor.matmul(ps[:], wsb[:], xt[:, j, :], start=True, stop=True)
            t1 = a1[0:64, i, 1 : 1 + W384]
            if edge:
                if j % 2 == 0:
                    # exact zero on invalid rows: relu(ps*rm + b*rm)
                    nc.scalar.activation(t1, ps[:], RELU,
                                         bias=actb[:, i : i + 1],
                                         scale=acts[:, i : i + 1])
                else:
                    nc.vector.tensor_scalar(t1, ps[:], bsb[:], 0.0, ADD, MAX)
                    vrows.append(j)
            else:
                if j % 2 == 0:
                    nc.scalar.activation(t1, ps[:], RELU, bias=bsb[:])
                else:
                    nc.vector.tensor_scalar(t1, ps[:], bsb[:], 0.0, ADD, MAX)
        if edge and vrows:
            # batched validity multiply for the Vector-activated rows
            # (contiguous odd rows handled as strided [64, n, 2, 384] view
            #  is not expressible; use two half-band ops on V and G)
            r0, r1 = vrows[0], vrows[-1] + 1
            mid = (r0 + r1) // 2
            for (ra, rb2), eng in (((r0, mid), nc.vector),
                                   ((mid, r1), nc.gpsimd)):
                if rb2 > ra:
                    eng.tensor_mul(
                        a1[0:64, i0 + ra : i0 + rb2, 1 : 1 + W384],
                        a1[0:64, i0 + ra : i0 + rb2, 1 : 1 + W384],
                        rm[0:64, i0 + ra : i0 + rb2].unsqueeze(-1)
                        .broadcast_to([64, rb2 - ra, W384]))
        # K-pack copy: partitions 64-127 hold x shifted left by one column
        # (value at col j = x[j]); col 385 already zeroed above. Sync ring:
        # a scalar-ring trigger here would head-of-line block the next
        # block's activations behind this block's Vector/GpSimd muls.
        nc.sync.dma_start(a1[64:128, i0 : i0 + nr, 0 : A1_WP - 1],
                          a1[0:64, i0 : i0 + nr, 1:A1_WP])

    # ---------------- L2 ----------------
    a2 = bufs["A2"]
    wsb2 = pools["const"].tile([128, 3, 64], F16)
    nc.sync.dma_start(wsb2[:], params["w2p"][:])
    wsr2 = pools["const"].tile([128, 3, 64], F16)
    nc.sync.dma_start(wsr2[:], params["w2r"][:])
    bsb2 = pools["const"].tile([64, 1], F32)
    nc.sync.dma_start(bsb2[:], params["b2"][:])
    rm2 = pools["rm192"]

    QB = 8  # A2 rows per block
    with tc.tile_pool(name="out2", bufs=3) as p_out2:
     for q0 in range(0, ROWS192["A2"], QB):
        nq = min(QB, ROWS192["A2"] - q0)
        mb = p_out2.tile([64, QB, WP], F16, tag="m2b")
        nc.gpsimd.memset(mb[:, :nq, 0:PAD], 0.0)
        nc.gpsimd.memset(mb[:, :nq, WP - PAD : WP], 0.0)
        for q in range(q0, q0 + nq):
            o2 = p_out2.tile([64, 2, W384], F16, tag="o2")
            for r in range(2):
                ps = pools["psum"].tile([64, W384], F32, tag="ps")
                for ai in range(3):
                    row = 2 * q + r + ai  # a1 row index (a = ai-1, +1 halo)
                    nc.tensor.matmul(ps[:], wsb2[:, ai, :],
                                     a1[:, row, 0:W384],
                                     start=(ai == 0), stop=False)
                    nc.tensor.matmul(ps[:], wsr2[:, ai, :],
                                     a1[:, row, 2 : 2 + W384],
                                     start=False, stop=(ai == 2))
                nc.scalar.activation(o2[:, r, :], ps[:], RELU, bias=bsb2[:])
            o2v = o2[:].rearrange("p r (c t) -> p r c t", t=2)
            cm = p_out2.tile([64, 2, W192], F16, tag="cm2")
            nc.vector.tensor_tensor(cm[:, 0, :], o2v[:, 0, :, 0], o2v[:, 0, :, 1], MAX)
            nc.vector.tensor_tensor(cm[:, 1, :], o2v[:, 1, :, 0], o2v[:, 1, :, 1], MAX)
            mp = mb[:, q - q0, PAD : PAD + W192]
            nc.vector.tensor_tensor(mp, cm[:, 0, :], cm[:, 1, :], MAX)
            if q < 20 or q >= ROWS192["A2"] - 20:
                nc.gpsimd.tensor_mul(
                    mp, mp, rm2[0:64, q : q + 1].broadcast_to([64, W192]))
        # dual write: main copy + column-shifted K-pack copy for L3
        nc.scalar.dma_start(a2[0:64, q0 : q0 + nq, :], mb[:, :nq, :])
        nc.scalar.dma_start(a2[64:128, q0 : q0 + nq, 0 : WP - 1],
                            mb[:, :nq, 1:WP])
        nc.scalar.dma_start(a2[64:128, q0 : q0 + nq, WP - 1 : WP],
                            mb[:, :nq, 0:1])


def _load_w192(nc, pools, params, widx, cin, cout, pool):
    """Load a 192-grid layer's weights+bias; DMAs split per 3 taps."""
    nci = (cin + 127) // 128
    nco = (cout + 127) // 128
    wts = []
    for ci in range(nci):
        p = min(128, cin - ci * 128)
        wt = pool.tile([p, 9, cout], F16, name=f"w{widx}_{ci}",
                       tag=f"w{widx}_{ci}")
        for t0 in range(0, 9, 3):
            nc.sync.dma_start(wt[:, t0 : t0 + 3, :],
                              params[f"w{widx}"][ci * 128 : ci * 128 + p,
                                                 t0 : t0 + 3])
        wts.append(wt)
    bsb = pools["const"].tile([min(cout, 128), nco], F32, name=f"bsb{widx}")
    nc.sync.dma_start(bsb[:], params[f"b{widx}"][:])
    return wts, bsb


def _load_w3(nc, pools, params, pool):
    """L3's K-packed weights: [128, 3, 128] pair-tap + zero-padded b=+1."""
    w3p = pool.tile([128, 3, 128], F16, name="w3p", tag="w3p")
    nc.sync.dma_start(w3p[:], params["w3p"][:])
    w3r = pool.tile([128, 3, 128], F16, name="w3r", tag="w3r")
    nc.sync.dma_start(w3r[:], params["w3r"][:])
    bsb = pools["const"].tile([128, 1], F32, name="bsb3")
    nc.sync.dma_start(bsb[:], params["b3"][:])
    return [w3p, w3r], bsb


def _emit_conv192(nc, tc, pools, bufs, params, src, dst, cin, cout, dils,
                  pool_s, widx, wts, bsb):
    """Generic 192-grid conv layer.

    Graph layers (len(dils)==2) use mask-gathered single-dilation inputs:
    per tap, Scalar engine copies the dilation-1 view into a fresh tile and
    DVE copy_predicated overwrites mask-1 pixels with the dilation-s view.
    Activations are deferred one pair to keep engines pipelined. Optional
    irregular pooling (pool_s) is fused into the output path.
    L3 (src A2) K-packs column taps: 6 K=128 matmuls per row-pair."""
    sdram, ddram = bufs[src], bufs[dst]
    rows_out = ROWS192[dst]
    h_out = HALO192[dst]
    graph = len(dils) == 2
    l3pack = src == "A2"
    dm = max(dils) if graph else 1
    s2 = dils[1] if graph else 1
    nci = (cin + 127) // 128
    nco = (cout + 127) // 128
    np_in = 128 if l3pack else None  # partitions to load for packed input
    off = 20 - h_out  # slice offset into A2-grid masks
    is_out = dst == "OUT"
    act_dt = F16
    rm = pools["rm192"]
    m2b = pools["m2_u8"]
    dst_c0 = 0 if is_out else PAD
    dst_wp = W192 if is_out else WP
    grp = pool_s if pool_s else 2  # rows per pool group

    pending = []  # deferred work closures

    def flush():
        while pending:
            pending.pop(0)()

    rb_blk = 4 if is_out else RB
    with tc.tile_pool(name=f"xin{widx}", bufs=2) as p_xin, \
         tc.tile_pool(name=f"g{widx}", bufs=20 if cin <= 256 else 16) as p_g, \
         tc.tile_pool(name=f"ob{widx}", bufs=3 if nco <= 2 else 2) as p_ob:
     for j0 in range(0, rows_out, rb_blk):
        rb = min(rb_blk, rows_out - j0)
        xts = []
        for ci in range(nci):
            p = np_in or min(128, cin - ci * 128)
            xt = p_xin.tile([p, rb_blk + 2 * dm, WP], F16, tag=f"xin{ci}")
            nrows = rb + 2 * dm
            half = (nrows + 1) // 2
            nc.sync.dma_start(xt[:, :half, :],
                              sdram[ci * 128 : ci * 128 + p, j0 : j0 + half, :])
            nc.sync.dma_start(xt[:, half:nrows, :],
                              sdram[ci * 128 : ci * 128 + p,
                                    j0 + half : j0 + nrows, :])
            xts.append(xt)
        obs = []
        for co in range(nco):
            pco = min(128, cout - co * 128)
            ob = p_ob.tile([pco, rb_blk, dst_wp], act_dt, tag=f"ob{co}")
            if not is_out:
                nc.gpsimd.memset(ob[:, :rb, 0:PAD], 0.0)
                nc.gpsimd.memset(ob[:, :rb, WP - PAD : WP], 0.0)
            obs.append(ob)
        # Shared gathers: one [p, gq, 192] gathered tile per (ci, tap) feeds
        # gq/2 row-pairs; center tap needs no gather. s=2 layers gather the
        # whole 8-row block (halves per-op init costs, which dominate the
        # ScalarE SBUF-src errata); s=4 layers stay at 4 rows (SBUF).
        gq = 8 if (graph and s2 == 2) else 4
        for q0 in range(0, rb, gq):
            nq = min(gq, rb - q0)
            gts = {}
            if graph:
                for ci in range(nci):
                    p = min(128, cin - ci * 128)
                    for ti, (a, b) in enumerate(TAPS):
                        if a == 0 and b == 0:
                            continue
                        g = p_g.tile([p, gq, W192], F16, tag="g")
                        src = xts[ci][:, q0 + dm + a : q0 + dm + a + nq,
                                      PAD + b : PAD + b + W192]
                        if b % 2:
                            # odd col offset: 2B-misaligned; SBUF->SBUF DMA
                            # on the sync ring (frees Scalar; triggers run
                            # ahead of compute so the copy lands early)
                            nc.sync.dma_start(g[:, :nq, :], src)
                        else:
                            # 4B-aligned: cheap DVE copy as int32 pairs
                            nc.vector.tensor_copy(g[:, :nq, :].bitcast(I32),
                                                  src.bitcast(I32))
                        # predicated overwrite as int32 pairs: adjacent pixel
                        # pairs share one mask value (4-wide mask blocks)
                        nc.vector.copy_predicated(
                            g[:, :nq, :].bitcast(I32),
                            m2b[:p, off + j0 + q0 : off + j0 + q0 + nq, :],
                            xts[ci][:, q0 + dm + a * s2 : q0 + dm + a * s2 + nq,
                                    PAD + b * s2 : PAD + b * s2 + W192]
                            .bitcast(I32))
                        gts[(ci, ti)] = g
            for g0 in range(q0, q0 + nq, 2):
                j = j0 + g0
                pss = [pools["psum"].tile([min(128, cout - co * 128), 2 * W192],
                                          F32, tag="ps",
                                          name=f"ps{widx}_{j}_{co}")
                       for co in range(nco)]
                if l3pack:
                    # 6 K=128 matmuls: 3 row-taps x (col pair, col +1)
                    for ai in range(3):
                        for ki in range(2):
                            cofs = PAD - 1 if ki == 0 else PAD + 1
                            rhs = xts[0][:, g0 + dm + ai - 1 :
                                         g0 + dm + ai + 1,
                                         cofs : cofs + W192]
                            nc.tensor.matmul(
                                pss[0][:, :], wts[ki][:, ai, :], rhs,
                                start=(ai == 0 and ki == 0),
                                stop=(ai == 2 and ki == 1))
                else:
                    for ci in range(nci):
                        for ti, (a, b) in enumerate(TAPS):
                            if graph and not (a == 0 and b == 0):
                                g = gts[(ci, ti)]
                                rhs = g[:, g0 - q0 : g0 - q0 + 2, :]
                            else:
                                rhs = xts[ci][:, g0 + dm + a : g0 + dm + a + 2,
                                              PAD + b : PAD + b + W192]
                            for co in range(nco):
                                pco = min(128, cout - co * 128)
                                nc.tensor.matmul(
                                    pss[co][:pco, :],
                                    wts[ci][:, ti, co * 128 : co * 128 + pco],
                                    rhs,
                                    start=(ci == 0 and ti == 0),
                                    stop=(ci == nci - 1 and ti == 8))
                def act_fn(j=j, j0=j0, g0=g0, pss=pss, obs=obs, rb=rb):
                    last_in_grp = (g0 + 2) % grp == 0 or g0 + 2 >= rb
                    for co in range(nco):
                        pco = min(128, cout - co * 128)
                        t1 = obs[co][:pco, g0 : g0 + 2, dst_c0 : dst_c0 + W192]
                        psv = pss[co][:pco, :].rearrange("p (r w) -> p r w",
                                                         w=W192)
                        nc.scalar.activation(t1, psv, RELU,
                                             bias=bsb[:pco, co : co + 1])
                        if j < h_out or j + 2 > rows_out - h_out:
                            nc.gpsimd.tensor_mul(
                                t1, t1,
                                rm[:pco, off + j : off + j + 2].unsqueeze(-1)
                                .broadcast_to([pco, 2, W192]))
                        if pool_s and last_in_grp:
                            jg = g0 + 2 - grp
                            tg = obs[co][:pco, jg : jg + grp,
                                         dst_c0 : dst_c0 + W192]
                            _emit_pool(nc, pools, tg, m2b, rm, pco, grp,
                                       pool_s, off + j0 + jg)

                pending.append(act_fn)
                if len(pending) > 1:
                    pending.pop(0)()

        def dma_fn(j0=j0, rb=rb, obs=obs):
            for co in range(nco):
                pco = min(128, cout - co * 128)
                nc.scalar.dma_start(
                    ddram[co * 128 : co * 128 + pco, j0 : j0 + rb, :],
                    obs[co][:pco, :rb, :])

        pending.append(dma_fn)
     flush()


def _emit_pool(nc, pools, tg, m2b, rm, pco, grp, pool_s, moff):
    """Fused irregular pool (block-max + replicate where mask) on tg
    [pco, grp, 192] in SBUF, grp == pool_s."""
    if pool_s == 2:
        tv = tg.rearrange("p r (c t) -> p r c t", t=2)
        cm = pools["pscr"].tile([pco, 2, W192 // 2], F16, tag="pcm")
        nc.vector.tensor_tensor(cm[:], tv[:, :, :, 0], tv[:, :, :, 1], MAX)
        bm = pools["pscr"].tile([pco, W192 // 2], F16, tag="pbm")
        nc.vector.tensor_tensor(bm[:], cm[:, 0, :], cm[:, 1, :], MAX)
        rep = pools["pscr"].tile([pco, 2, W192], F16, tag="prep")
        nc.vector.tensor_copy(
            rep[:], bm[:].unsqueeze(1).unsqueeze(-1)
            .broadcast_to([pco, 2, W192 // 2, 2]))
        nc.vector.copy_predicated(
            tg.bitcast(I32), m2b[:pco, moff : moff + 2, :],
            rep[:].bitcast(I32))
    elif pool_s == 4:
        tv = tg.rearrange("p r (c t) -> p r c t", t=4)
        c1 = pools["pscr"].tile([pco, 4, W192 // 4], F16, tag="pc1")
        c2 = pools["pscr"].tile([pco, 4, W192 // 4], F16, tag="pc2")
        nc.vector.tensor_tensor(c1[:], tv[:, :, :, 0], tv[:, :, :, 1], MAX)
        nc.vector.tensor_tensor(c2[:], tv[:, :, :, 2], tv[:, :, :, 3], MAX)
        nc.vector.tensor_tensor(c1[:], c1[:], c2[:], MAX)
        r1 = pools["pscr"].tile([pco, W192 // 4], F16, tag="pr1")
        r2 = pools["pscr"].tile([pco, W192 // 4], F16, tag="pr2")
        nc.vector.tensor_tensor(r1[:], c1[:, 0, :], c1[:, 1, :], MAX)
        nc.vector.tensor_tensor(r2[:], c1[:, 2, :], c1[:, 3, :], MAX)
        nc.vector.tensor_tensor(r1[:], r1[:], r2[:], MAX)
        rep = pools["pscr"].tile([pco, 4, W192], F16, tag="prep4")
        nc.vector.tensor_copy(
            rep[:], r1[:].unsqueeze(1).unsqueeze(-1)
            .broadcast_to([pco, 4, W192 // 4, 4]))
        nc.vector.copy_predicated(
            tg.bitcast(I32), m2b[:pco, moff : moff + 4, :],
            rep[:].bitcast(I32))


def build_program():
    nc = bacc.Bacc()
    params = {}
    params["x1col"] = nc.declare_dram_parameter(
        "x1col", [27, A1_ROWS, W384], F16, isOutput=False)
    params["w1"] = nc.declare_dram_parameter("w1", [27, 64], F16, isOutput=False)
    for i, (ci, co) in enumerate(_CHANS):
        if i >= 3:  # w4..w10 standard layout; w2/w3 packed separately
            params[f"w{i + 1}"] = nc.declare_dram_parameter(
                f"w{i + 1}", [ci, 9, co], F16, isOutput=False)
        params[f"b{i + 1}"] = nc.declare_dram_parameter(
            f"b{i + 1}", [min(co, 128), (co + 127) // 128], F32, isOutput=False)
    params["m2_u8"] = nc.declare_dram_parameter(
        "m2_u8", [128, ROWS192["A2"], W192 // 2], U8, isOutput=False)
    params["b1r"] = nc.declare_dram_parameter(
        "b1r", [64, A1_ROWS], F32, isOutput=False)
    params["rm384f"] = nc.declare_dram_parameter(
        "rm384f", [64, A1_ROWS], F32, isOutput=False)
    params["rm384h"] = nc.declare_dram_parameter(
        "rm384h", [64, A1_ROWS], F16, isOutput=False)
    params["rm192"] = nc.declare_dram_parameter(
        "rm192", [128, ROWS192["A2"]], F16, isOutput=False)

    params["w2p"] = nc.declare_dram_parameter("w2p", [128, 3, 64], F16,
                                              isOutput=False)
    params["w2r"] = nc.declare_dram_parameter("w2r", [128, 3, 64], F16,
                                              isOutput=False)
    params["w3p"] = nc.declare_dram_parameter("w3p", [128, 3, 128], F16,
                                              isOutput=False)
    params["w3r"] = nc.declare_dram_parameter("w3r", [128, 3, 128], F16,
                                              isOutput=False)
    bufs = {"X1": params["x1col"]}
    for name, cc in (("A2", 128), ("A3", 128), ("A4", 128), ("A5", 256),
                     ("A6", 256), ("A7", 256), ("A8", 512), ("A9", 512)):
        bufs[name] = nc.dram_tensor(name, [cc, ROWS192[name], WP], F16)
    bufs["OUT"] = nc.declare_dram_parameter(
        "out", [512, ROWS192["OUT"], W192], F16, isOutput=True)

    with tile.TileContext(nc) as tc:
        from contextlib import ExitStack
        with ExitStack() as ctx:
            pools = {}
            for name, kw in (
                ("const", dict(bufs=1)),
                ("pscr", dict(bufs=2)),
                ("psum", dict(bufs=8, space="PSUM")),
            ):
                pools[name] = ctx.enter_context(tc.tile_pool(name=name, **kw))
            pools["rm192"] = pools["const"].tile([128, ROWS192["A2"]], F16,
                                                 name="rm192_t", tag="rm192")
            nc.sync.dma_start(pools["rm192"][:], params["rm192"][:])

            with tc.tile_pool(name="a1", bufs=1) as p_a1:
                pools["a1"] = p_a1
                _emit_l12(nc, tc, pools, bufs, params)

            # half-res mask (first used by L4's fused pool / L5 gathers)
            pools["m2_u8"] = pools["const"].tile(
                [128, ROWS192["A2"], W192 // 2], U8, name="m2_u8_t",
                tag="m2_u8")
            nc.sync.dma_start(pools["m2_u8"][:], params["m2_u8"][:])

            # Weight loads are emitted at layer start; since the sync ring
            # carries only loads (stores go via the scalar ring), each
            # layer's weights land during the previous layer's compute.
            def emit(lay, pool):
                widx, cin, cout = lay[-1], lay[2], lay[3]
                if widx == 3:
                    wts, bsb = _load_w3(nc, pools, params, pool)
                else:
                    wts, bsb = _load_w192(nc, pools, params, widx, cin,
                                          cout, pool)
                _emit_conv192(nc, tc, pools, bufs, params, *lay, wts, bsb)

            with tc.tile_pool(name="wresA", bufs=1) as p_wa:
                for lay in LAYERS[:5]:
                    emit(lay, p_wa)
            with tc.tile_pool(name="wresB2", bufs=1) as p_wb2:
                with tc.tile_pool(name="wresB1", bufs=1) as p_wb1:
                    emit(LAYERS[5], p_wb1)  # L8
                for lay in LAYERS[6:]:
                    emit(lay, p_wb2)
    nc.compile()
    return nc


# ---------------------------------------------------------------- host side

F16_NP = np.float16


def _upsample_mask(m48):
    return np.repeat(np.repeat(m48, 4, axis=0), 4, axis=1)


def make_core_inputs(inputs, core):
    b, s = core // 4, core % 4
    r0, R0 = 48 * s, 96 * s
    x = np.asarray(inputs["batch"][b], np.float32)  # [3, 384, 384]

    canvas = np.zeros((3, CANVAS_ROWS, W384 + 2), np.float32)
    lo, hi = R0 - 42, R0 + 138
    clo, chi = max(lo, 0), min(hi, W384)
    canvas[:, clo - lo : chi - lo, 1 : 1 + W384] = x[:, clo:chi, :]

    x1col = np.empty((27, A1_ROWS, W384), np.float32)
    for t, (a, bb) in enumerate(TAPS):
        x1col[3 * t : 3 * t + 3] = canvas[:, 1 + a : 1 + a + A1_ROWS,
                                          1 + bb : 1 + bb + W384]

    m192 = _upsample_mask(np.asarray(inputs["pooling_mask"][b, 0]))  # [192,192]
    mbuf = np.zeros((ROWS192["A2"], W192), np.uint8)
    mlo, mhi = r0 - 20, r0 + 68
    cmlo, cmhi = max(mlo, 0), min(mhi, W192)
    mbuf[cmlo - mlo : cmhi - mlo] = m192[cmlo:cmhi].astype(np.uint8)

    # A1 row i is valid iff image row R0-41+i is inside the image
    rmA1 = ((np.arange(A1_ROWS) + R0 - 41 >= 0)
            & (np.arange(A1_ROWS) + R0 - 41 < W384)).astype(np.float32)
    rm192 = ((np.arange(ROWS192["A2"]) + r0 - 20 >= 0)
             & (np.arange(ROWS192["A2"]) + r0 - 20 < W192)).astype(np.float32)

    b1 = np.asarray(inputs["b1"], np.float32)
    im = {
        "x1col": x1col.astype(F16_NP),
        "m2_u8": np.broadcast_to(np.ascontiguousarray(mbuf[:, 0::2]),
                                 (128, mbuf.shape[0], mbuf.shape[1] // 2)
                                 ).copy(),
        "b1r": np.ascontiguousarray(np.outer(b1, rmA1)).astype(np.float32),
        "rm384f": np.broadcast_to(rmA1, (64, A1_ROWS)).astype(np.float32).copy(),
        "rm384h": np.broadcast_to(rmA1, (64, A1_ROWS)).astype(F16_NP).copy(),
        "rm192": np.broadcast_to(rm192, (128, ROWS192["A2"])).astype(F16_NP),
    }
    w1 = np.asarray(inputs["w1"], np.float32)  # [64, 3, 3, 3]
    w1r = np.empty((27, 64), np.float32)
    for t, (a, bb) in enumerate(TAPS):
        w1r[3 * t : 3 * t + 3] = w1[:, :, a + 1, bb + 1].T
    im["w1"] = w1r.astype(F16_NP)
    for i in range(4, 11):
        w = np.asarray(inputs[f"w{i}"], np.float32)  # [O, I, 3, 3]
        im[f"w{i}"] = np.ascontiguousarray(
            w.transpose(1, 2, 3, 0).reshape(w.shape[1], 9, w.shape[0])
        ).astype(F16_NP)
    w2 = np.asarray(inputs["w2"], np.float32)  # [64, 64, 3, 3]
    w2p = np.empty((128, 3, 64), np.float32)
    for ai in range(3):
        w2p[0:64, ai] = w2[:, :, ai, 0].T   # b = -1 taps, low partitions
        w2p[64:128, ai] = w2[:, :, ai, 1].T  # b = 0 taps, high partitions
    im["w2p"] = w2p.astype(F16_NP)
    w2r = np.zeros((128, 3, 64), np.float32)
    w2r[0:64] = w2.transpose(1, 2, 0, 3)[:, :, :, 2]
    im["w2r"] = w2r.astype(F16_NP)
    w3 = np.asarray(inputs["w3"], np.float32)  # [128, 64, 3, 3]
    w3p = np.empty((128, 3, 128), np.float32)
    for ai in range(3):
        w3p[0:64, ai] = w3[:, :, ai, 0].T   # b = -1 taps, low partitions
        w3p[64:128, ai] = w3[:, :, ai, 1].T  # b = 0 taps, high partitions
    im["w3p"] = w3p.astype(F16_NP)
    w3r = np.zeros((128, 3, 128), np.float32)
    w3r[0:64] = w3.transpose(1, 2, 0, 3)[:, :, :, 2]
    im["w3r"] = w3r.astype(F16_NP)
    for i in range(1, 11):
        bv = np.asarray(inputs[f"b{i}"], np.float32)
        im[f"b{i}"] = np.ascontiguousarray(bv.reshape(-1, min(bv.size, 128)).T)
    return im


_NC_CACHE = []


def _get_program():
    if not _NC_CACHE:
        _NC_CACHE.append(build_program())
    return _NC_CACHE[0]


def kernel(**inputs):
    nc = _get_program()
    in_maps = [make_core_inputs(inputs, c) for c in range(8)]
    res = run_bass_kernel_spmd(nc, in_maps, list(range(8)))
    out = np.empty((2, 512, W192, W192), np.float32)
    for c in range(8):
        b, s = c // 4, c % 4
        out[b, :, 48 * s : 48 * s + 48, :] = res.results[c]["out"]
    return out
